# revision 8
# baseline (speedup 1.0000x reference)
"""Multi-head attention (B=4, S=2048, D=1024, H=16, Dh=64) on 8 NeuronCores.

Sharding: core c handles batch b=c//2 and head-group g=c%2 (8 heads).
wq/wk/wv column-parallel, wo row-parallel; host sums the two partial
wo-products per batch and adds bo.

v3 changes vs v2 (346us):
- startup: big critical DMAs (xq0/xk0/wq) issued first across all three
  DMA queues (sync/scalar HWDGE + gpsimd SWDGE); constants moved behind
  them. Cuts the 26us PE-idle head to ~13us.
- softmax denominator broadcast moved off the PE: reciprocal of the
  [1,512] ones-row directly from PSUM, then gpsimd partition_broadcast.
  (was a 13us fp32 ones-matmul on the tensor engine)
- V bias moved off the PE: bv pre-broadcast to [128, DG] once, added on
  the DVE during the V PSUM evacuation (was a K=1 matmul per V tile).
- wo accumulates all 4 head pairs into ONE PSUM tile -> single out
  tensor (half the evac + output-DMA bytes), output DMAs alternate
  sync/gpsimd queues.
- diagonal-slot exps merged into one ACT instruction where profitable.
"""

import sys

sys.path.insert(0, "/opt/trn_rl_repo")

import ml_dtypes
import numpy as np

import concourse.bass as bass  # noqa: F401
import concourse.bacc as bacc
import concourse.tile as tile
import concourse.mybir as mybir
from concourse.bass_utils import run_bass_kernel_spmd

F32 = mybir.dt.float32
F32R = mybir.dt.float32r
BF16 = mybir.dt.bfloat16
AF = mybir.ActivationFunctionType
BF = ml_dtypes.bfloat16

B, S, D = 4, 2048, 1024
H, DH = 16, 64
HG = 8  # heads per core
DG = HG * DH  # 512 out-dims per core

_PROGRAM = None
LAST_RESULTS = None  # for test.py introspection


def _build_program():
    nc = bacc.Bacc("TRN2", target_bir_lowering=False, debug=False)

    xq_t = nc.dram_tensor("xq_t", [4, 128, 8 * 512], BF16, kind="ExternalInput")
    xk_t = nc.dram_tensor("xk_t", [4, 128, 8 * 512], BF16, kind="ExternalInput")
    xv_t = nc.dram_tensor("xv_t", [16, 128, 8 * 128], BF16, kind="ExternalInput")
    wq_t = nc.dram_tensor("wq_t", [128, 8 * DG], BF16, kind="ExternalInput")
    wk_t = nc.dram_tensor("wk_t", [128, 8 * DG], BF16, kind="ExternalInput")
    wv_t = nc.dram_tensor("wv_t", [128, 8 * DG], BF16, kind="ExternalInput")
    wo_t = nc.dram_tensor("wo_t", [DG, D], BF16, kind="ExternalInput")
    bq_c = nc.dram_tensor("bq_c", [128, 4], F32, kind="ExternalInput")
    bk_c = nc.dram_tensor("bk_c", [128, 4], F32, kind="ExternalInput")
    bv_r = nc.dram_tensor("bv_r", [1, DG], BF16, kind="ExternalInput")
    ones_b = nc.dram_tensor("ones_b", [1, 128], BF16, kind="ExternalInput")
    ones_f = nc.dram_tensor("ones_f", [1, 128], BF16, kind="ExternalInput")
    ones8 = nc.dram_tensor("ones8", [128, 8], BF16, kind="ExternalInput")
    tri01 = nc.dram_tensor("tri01", [128, 128], BF16, kind="ExternalInput")
    out0 = nc.dram_tensor("out0", [S, D], BF16, kind="ExternalOutput")

    with tile.TileContext(nc) as tc:
        with (
            nc.allow_low_precision(reason="bf16 attention pipeline"),
            tc.tile_pool(name="persist", bufs=1) as pers,
        ):
            # ---- persistent tiles ----
            qT = [pers.tile([128, S], BF16, name=f"qT{i}") for i in range(4)]
            kT = [pers.tile([128, S], BF16, name=f"kT{i}") for i in range(4)]
            # v tiles: [128 s, 8 heads x (64 v + 1 ones)]
            vt = [pers.tile([128, HG * 65], BF16, name=f"v{i}") for i in range(16)]
            aout = [pers.tile([128, S], BF16, name=f"ao{i}") for i in range(4)]
            tri_sb = pers.tile([128, 128], BF16, name="tri_sb")
            ones_bf = pers.tile([1, 128], BF16, name="ones_bf")
            ones_fr = pers.tile([65, 128], BF16, name="ones_fr")
            bq_sb = pers.tile([128, 4], F32, name="bq")
            bk_sb = pers.tile([128, 4], F32, name="bk")
            bv_sb = pers.tile([1, DG], BF16, name="bv")
            bv_bc = pers.tile([128, DG], BF16, name="bv_bc")
            wq_sb = pers.tile([128, 8 * DG], BF16, name="wq_sb")
            wk_sb = pers.tile([128, 8 * DG], BF16, name="wk_sb")
            wv_sb = pers.tile([128, 8 * DG], BF16, name="wv_sb")
            wo_sb = [
                pers.tile([128, D], BF16, name=f"wo{c}") for c in range(4)
            ]

            # PSUM budget (8 banks): scores 2x[128,1024]=4, po 2x[65,512]=2,
            # pw 2x[128,512]=2 (proj evac / wo)
            pp = tc.alloc_tile_pool(name="pp", bufs=2, space="PSUM")
            with (
                tc.tile_pool(name="xb", bufs=5) as xp,
                tc.tile_pool(name="at", bufs=6) as ap_,
                tc.tile_pool(name="sm", bufs=6) as sm,
                tc.tile_pool(name="ob", bufs=4) as obp,
            ):
                # ---- startup DMAs: the first projection matmuls need only
                # wq/xq0 halves, so those go first (sync + gpsimd rings in
                # parallel); xk0/wk behind them (k-proj is emitted after all
                # of round-0 q-proj), wv last. Halves let the k8-accumulation
                # start as soon as the first 512KB lands.
                xq_big0 = xp.tile([128, 8 * 512], BF16, tag="xb", name="xq_big")
                xk_big0 = xp.tile([128, 8 * 512], BF16, tag="xb", name="xk_big")
                xv_early = []
                HC = 4 * 512
                nc.scalar.dma_start(out=wq_sb[:, 0:HC], in_=wq_t[:, 0:HC])
                nc.sync.dma_start(out=xq_big0[:, 0:HC], in_=xq_t[0, :, 0:HC])
                nc.gpsimd.dma_start(out=wq_sb[:, HC:], in_=wq_t[:, HC:])
                nc.sync.dma_start(out=xq_big0[:, HC:], in_=xq_t[0, :, HC:])
                nc.scalar.dma_start(out=bq_sb[:], in_=bq_c[:])
                nc.scalar.dma_start(out=bk_sb[:], in_=bk_c[:])
                nc.scalar.dma_start(out=xk_big0[:, 0:HC], in_=xk_t[0, :, 0:HC])
                nc.scalar.dma_start(out=xk_big0[:, HC:], in_=xk_t[0, :, HC:])
                nc.gpsimd.dma_start(out=wk_sb[:, 0:HC], in_=wk_t[:, 0:HC])
                nc.gpsimd.dma_start(out=wk_sb[:, HC:], in_=wk_t[:, HC:])
                nc.scalar.dma_start(out=wv_sb[:, 0:HC], in_=wv_t[:, 0:HC])
                nc.scalar.dma_start(out=wv_sb[:, HC:], in_=wv_t[:, HC:])
                # first four V activation chunks early on the sync ring
                for s4 in range(4):
                    xvb = xp.tile([128, 8 * 128], BF16, tag="xvb", name="xv_big")
                    xv_early.append(xvb)
                    nc.sync.dma_start(out=xvb[:], in_=xv_t[s4])
                nc.sync.dma_start(out=ones_bf[:], in_=ones_b[:])
                nc.sync.dma_start(out=ones_fr[64:65, :], in_=ones_f[:])
                nc.sync.dma_start(out=tri_sb[:], in_=tri01[:])
                nc.sync.dma_start(out=bv_sb[:], in_=bv_r[:])
                nc.gpsimd.partition_broadcast(bv_bc[:], bv_sb[:])
                # the per-head "ones" column of every v tile never changes:
                # fill them all up front (cheap DVE strided writes)
                ones8_sb = pers.tile([128, 8], BF16, name="ones8_sb")
                nc.sync.dma_start(out=ones8_sb[:], in_=ones8[:])
                for s in range(16):
                    v3s = vt[s].rearrange("p (h x) -> p h x", x=65)
                    nc.vector.tensor_copy(v3s[:, :, 64:65], ones8_sb[:].unsqueeze(2))
                # warm the ACT exp table (~2.7us) before the first real exp
                dummy = pers.tile([1, 128], BF16, name="dummy")
                nc.scalar.activation(dummy[:], ones_bf[:], AF.Exp)

                xqk_cache = {0: (xq_big0, xk_big0)}

                def emit_proj_qk(n, ms=(0, 1, 2, 3), which="qk"):
                    if n in xqk_cache:
                        xq_big, xk_big = xqk_cache[n]
                    else:
                        xq_big = xp.tile(
                            [128, 8 * 512], BF16, tag="xb", name="xq_big"
                        )
                        xk_big = xp.tile(
                            [128, 8 * 512], BF16, tag="xb", name="xk_big"
                        )
                        xqk_cache[n] = (xq_big, xk_big)
                        nc.sync.dma_start(out=xq_big[:], in_=xq_t[n])
                        nc.scalar.dma_start(out=xk_big[:], in_=xk_t[n])
                    parts = []
                    if "q" in which:
                        parts.append((wq_sb, xq_big, bq_sb, qT))
                    if "k" in which:
                        parts.append((wk_sb, xk_big, bk_sb, kT))
                    for m in ms:
                        for w_big, x_big, b_sb, dst in parts:
                            ps = pp.tile([128, 512], F32, tag="pw", bufs=2, name="ps1")
                            for k8 in range(8):
                                nc.tensor.matmul(
                                    ps[:],
                                    w_big[
                                        :,
                                        k8 * DG + m * 128 : k8 * DG + (m + 1) * 128,
                                    ],
                                    x_big[:, k8 * 512 : (k8 + 1) * 512],
                                    start=(k8 == 0),
                                    stop=(k8 == 7),
                                )
                            nc.scalar.activation(
                                dst[m][:, n * 512 : (n + 1) * 512],
                                ps[:],
                                AF.Identity,
                                bias=b_sb[:, m : m + 1],
                            )

                def emit_proj_v(s):
                    if s < 4:
                        xv_big = xv_early[s]
                    else:
                        xv_big = xp.tile(
                            [128, 8 * 128], BF16, tag="xvb", name="xv_big"
                        )
                        nc.sync.dma_start(out=xv_big[:], in_=xv_t[s])
                    ps = pp.tile([128, DG], F32, tag="pw", bufs=2, name="psv")
                    for k8 in range(8):
                        nc.tensor.matmul(
                            ps[:],
                            xv_big[:, k8 * 128 : (k8 + 1) * 128],
                            wv_sb[:, k8 * DG : (k8 + 1) * DG],
                            start=(k8 == 0),
                            stop=(k8 == 7),
                        )
                    v3 = vt[s].rearrange("p (h x) -> p h x", x=65)
                    nc.vector.tensor_add(
                        v3[:, :, 0:64],
                        ps[:].rearrange("p (h d) -> p h d", d=64),
                        bv_bc[:].rearrange("p (h d) -> p h d", d=64),
                    )

                def emit_attn(p, j, filler=None):
                    hA, hB = 2 * p, 2 * p + 1
                    nsk = 4 * j + 4
                    ps_oA = pp.tile([65, 512], F32, tag="po", bufs=2, name="ps_oA")
                    ps_oB = pp.tile([65, 512], F32, tag="po", bufs=2, name="ps_oB")
                    pending = []
                    # diagonal slots FIRST so the pair tail never waits on
                    # them; slot 4j has c0=0 so the first PV write still
                    # covers the full [0:512).
                    order = list(range(4 * j, nsk)) + list(range(0, 4 * j))
                    first_i = order[0]
                    for i in order:
                        koff = i - 4 * j
                        c0 = max(0, koff * 128) if koff >= 0 else 0
                        # both heads' scores in one 2-bank PSUM tile:
                        # head A at cols [0:512), head B at [512:1024).
                        # K=64 each, row-packed on the PE at row groups 0/64.
                        psAB = pp.tile([128, 1024], F32, tag="ps", bufs=2, name="psAB")
                        diag = koff >= 0
                        nc.tensor.matmul(
                            psAB[:, c0:512],
                            kT[p][0:64, i * 128 : (i + 1) * 128],
                            qT[p][0:64, j * 512 + c0 : (j + 1) * 512],
                            start=True,
                            stop=True,
                            tile_position=(0, 0),
                        )
                        nc.tensor.matmul(
                            psAB[:, 512 + c0 : 1024],
                            kT[p][64:128, i * 128 : (i + 1) * 128],
                            qT[p][64:128, j * 512 + c0 : (j + 1) * 512],
                            start=True,
                            stop=True,
                            tile_position=(64, 0),
                        )
                        # software-pipeline skew: retire PVs two iterations
                        # behind the exps so a PV never waits on its exp
                        while len(pending) >= 3:
                            pi, pc0, pat = pending.pop(0)
                            nc.tensor.matmul(
                                ps_oA[:, pc0:512],
                                vt[pi][:, hA * 65 : hA * 65 + 65],
                                pat[:, pc0:512],
                                start=(pi == first_i),
                                stop=False,
                            )
                            nc.tensor.matmul(
                                ps_oB[:, pc0:512],
                                vt[pi][:, hB * 65 : hB * 65 + 65],
                                pat[:, 512 + pc0 : 1024],
                                start=(pi == first_i),
                                stop=False,
                            )
                        atAB = ap_.tile([128, 1024], BF16, tag="at", name="atAB")
                        if c0 == 0:
                            nc.scalar.activation(
                                atAB[:], psAB[:], AF.Exp, scale=0.125
                            )
                        elif koff < 3 and not (p == 0 and j == 0):
                            # one merged exp over [c0:1024); the gap
                            # [512:512+c0) holds stale PSUM whose exp lands
                            # in an unread atAB region (skipped for the very
                            # first psAB allocations, which are uninit)
                            nc.scalar.activation(
                                atAB[:, c0:1024],
                                psAB[:, c0:1024],
                                AF.Exp,
                                scale=0.125,
                            )
                        else:
                            nc.scalar.activation(
                                atAB[:, c0:512], psAB[:, c0:512], AF.Exp, scale=0.125
                            )
                            nc.scalar.activation(
                                atAB[:, 512 + c0 : 1024],
                                psAB[:, 512 + c0 : 1024],
                                AF.Exp,
                                scale=0.125,
                            )
                        if diag:
                            # causal mask: zero the sub-diagonal triangle of
                            # the diag block after exp (was -1e9 on the PE)
                            nc.vector.tensor_mul(
                                atAB[:, c0 : c0 + 128],
                                atAB[:, c0 : c0 + 128],
                                tri_sb[:],
                            )
                            nc.vector.tensor_mul(
                                atAB[:, 512 + c0 : 512 + c0 + 128],
                                atAB[:, 512 + c0 : 512 + c0 + 128],
                                tri_sb[:],
                            )
                        pending.append((i, c0, atAB))
                        if filler:
                            emit_wo_item(*filler.pop(0))
                    while pending:
                        pi, pc0, pat = pending.pop(0)
                        last = not pending
                        nc.tensor.matmul(
                            ps_oA[:, pc0:512],
                            vt[pi][:, hA * 65 : hA * 65 + 65],
                            pat[:, pc0:512],
                            start=(pi == first_i),
                            stop=last,
                        )
                        nc.tensor.matmul(
                            ps_oB[:, pc0:512],
                            vt[pi][:, hB * 65 : hB * 65 + 65],
                            pat[:, 512 + pc0 : 1024],
                            start=(pi == first_i),
                            stop=last,
                        )
                    # normalize both heads; write bf16 straight into aout
                    for ps_o, hp in ((ps_oA, 0), (ps_oB, 64)):
                        den = sm.tile([65, 512], BF16, tag="den", bufs=2, name="den")
                        nc.vector.tensor_copy(den[64:65, :], ps_o[64:65, :])
                        ps_bc = pp.tile(
                            [64, 512], F32, tag="pw", bufs=2, name="ps_bc"
                        )
                        nc.tensor.matmul(
                            ps_bc[:],
                            ones_fr[64:65, 0:64],
                            den[64:65, :],
                            start=True,
                            stop=True,
                        )
                        rb = sm.tile([64, 512], F32, tag="rb", bufs=3, name="rb")
                        nc.vector.reciprocal_approx_fast(out=rb[:], in_=ps_bc[:])
                        dst = aout[p][hp : hp + 64, j * 512 : (j + 1) * 512]
                        if hp == 0:
                            nc.vector.tensor_mul(dst, ps_o[0:64, :], rb[:])
                        else:
                            tmp = sm.tile([64, 512], BF16, tag="tmp", bufs=3, name="tmp")
                            nc.vector.tensor_mul(tmp[:], ps_o[0:64, :], rb[:])
                            nc.gpsimd.dma_start(out=dst, in_=tmp[:])

                def emit_wo_item(s, n2, act_evac=False):
                    # accumulate all 4 head pairs into one PSUM tile
                    psw = pp.tile([128, 512], F32, tag="pw", bufs=2, name="psw")
                    for p4 in range(4):
                        nc.tensor.matmul(
                            psw[:],
                            aout[p4][:, s * 128 : (s + 1) * 128],
                            wo_sb[p4][:, n2 * 512 : (n2 + 1) * 512],
                            start=(p4 == 0),
                            stop=(p4 == 3),
                        )
                    ob = obp.tile([128, 512], BF16, tag="ob", name="ob")
                    if act_evac:
                        nc.scalar.copy(ob[:], psw[:])
                    else:
                        nc.vector.tensor_copy(ob[:], psw[:])
                    eng = nc.gpsimd if (2 * s + n2) % 2 else nc.sync
                    eng.dma_start(
                        out=out0[
                            s * 128 : (s + 1) * 128,
                            n2 * 512 : (n2 + 1) * 512,
                        ],
                        in_=ob[:],
                    )

                # ---- j-round synchronized emission ----
                # proj(n) and V(s) for round j+1 are emitted DURING round j
                # so their PE matmuls fill exp-paced attention bubbles and
                # their ACT evacuations land ahead of round j+1's exps.
                # output-projection items whose aout columns (s-tiles
                # 0..11 = j-chunks 0..2) are final before round 3 begins;
                # drained one per attention slot during round 3
                wo_fill = [(s, n2) for s in range(12) for n2 in range(2)]
                emit_proj_qk(0, ms=(0, 1), which="q")
                emit_proj_qk(0, ms=(0, 1), which="k")
                for s in range(4):
                    emit_proj_v(s)
                for j in range(4):
                    for p in range(4):
                        if p == 0 and j == 0:
                            emit_proj_qk(0, ms=(2, 3))
                        if p == 0 and j < 3:
                            emit_proj_qk(j + 1)
                        if p == 0 and j == 1:
                            # wo weights aren't needed until round 3
                            for c in range(4):
                                nc.gpsimd.dma_start(
                                    out=wo_sb[c][:],
                                    in_=wo_t[c * 128 : (c + 1) * 128, :],
                                )
                        # V tiles pulled forward: their evacuation runs on the
                        # (idle) vector engine, so unlike projections this
                        # fill adds no load to the exp-pacing scalar engine
                        if j == 0 and p == 1:
                            for s in range(4, 8):
                                emit_proj_v(s)
                        if j == 0 and p == 3:
                            for s in range(8, 12):
                                emit_proj_v(s)
                        if j == 1 and p == 1:
                            for s in range(12, 16):
                                emit_proj_v(s)
                        if j == 3:
                            emit_attn(p, j, filler=wo_fill)
                        else:
                            emit_attn(p, j)
                for item in wo_fill:
                    emit_wo_item(*item)
                for s in range(12, 16):
                    emit_wo_item(s, 0, act_evac=True)
                    emit_wo_item(s, 1)

            pp.release()

    nc.compile()
    return nc


def _make_in_maps(query, key, value, wq, bq, wk, bk, wv, bv, wo):
    f32 = np.float32
    ones_b = np.ones((1, 128), BF)
    ones8 = np.ones((128, 8), BF)
    # causal 0/1 mask for the diagonal block in scores_T layout:
    # rows=sk_local, cols=sq_local; valid iff sq_local >= sk_local
    tri01 = np.triu(np.ones((128, 128), np.float32)).astype(BF)

    wqT = np.asarray(wq, f32).T.astype(BF)  # [D, D] (d, dq)
    wkT = np.asarray(wk, f32).T.astype(BF)
    wvT = np.asarray(wv, f32).T.astype(BF)
    woT = np.asarray(wo, f32).T.astype(BF)  # [dv, D]

    def chunked_w(wT):
        # [1024, 512] (d, dq) -> [128, 8*512]: partition p holds d = k*128+p
        return np.ascontiguousarray(
            wT.reshape(8, 128, DG).transpose(1, 0, 2).reshape(128, 8 * DG)
        )

    def chunked_x(xT, n_chunks, cw):
        # [1024, S] -> [n_chunks, 128, 8*cw]: chunk n, partition p holds
        # rows d = k*128+p of columns [n*cw, (n+1)*cw)
        x3 = xT.reshape(8, 128, S)
        return np.ascontiguousarray(
            np.stack(
                [
                    x3[:, :, n * cw : (n + 1) * cw]
                    .transpose(1, 0, 2)
                    .reshape(128, 8 * cw)
                    for n in range(n_chunks)
                ]
            )
        )

    in_maps = []
    for c in range(8):
        b, g = c // 2, c % 2
        sl = slice(g * DG, (g + 1) * DG)
        in_maps.append(
            {
                "xq_t": chunked_x(np.asarray(query[b], f32).T.astype(BF), 4, 512),
                "xk_t": chunked_x(np.asarray(key[b], f32).T.astype(BF), 4, 512),
                "xv_t": chunked_x(np.asarray(value[b], f32).T.astype(BF), 16, 128),
                "wq_t": chunked_w(wqT[:, sl]),
                "wk_t": chunked_w(wkT[:, sl]),
                "wv_t": chunked_w(wvT[:, sl]),
                "wo_t": np.ascontiguousarray(woT[sl, :]),
                "bq_c": np.ascontiguousarray(
                    np.asarray(bq, f32)[sl].reshape(4, 128).T
                ),
                "bk_c": np.ascontiguousarray(
                    np.asarray(bk, f32)[sl].reshape(4, 128).T
                ),
                "bv_r": np.asarray(bv, f32)[sl].reshape(1, DG).astype(BF),
                "ones_b": ones_b,
                "ones_f": np.ones((1, 128), BF),
                "ones8": ones8,
                "tri01": tri01,
            }
        )
    return in_maps


def kernel(query, key, value, mask, wq, bq, wk, bk, wv, bv, wo, bo):
    global _PROGRAM, LAST_RESULTS
    if _PROGRAM is None:
        _PROGRAM = _build_program()
    nc = _PROGRAM
    in_maps = _make_in_maps(query, key, value, wq, bq, wk, bk, wv, bv, wo)

    res = run_bass_kernel_spmd(nc, in_maps, core_ids=list(range(8)))
    LAST_RESULTS = res

    f32 = np.float32
    out = np.empty((B, S, D), f32)
    for b in range(B):
        out[b] = np.asarray(res.results[2 * b]["out0"], f32) + np.asarray(
            res.results[2 * b + 1]["out0"], f32
        )
    out += np.asarray(bo, f32)[None, None, :]
    return out


# revision 9
# speedup vs baseline: 1.0119x; 1.0119x over previous
"""Multi-head attention (B=4, S=2048, D=1024, H=16, Dh=64) on 8 NeuronCores.

Sharding: core c handles batch b=c//2 and head-group g=c%2 (8 heads).
wq/wk/wv column-parallel, wo row-parallel; host sums the two partial
wo-products per batch and adds bo.

v3 changes vs v2 (346us):
- startup: big critical DMAs (xq0/xk0/wq) issued first across all three
  DMA queues (sync/scalar HWDGE + gpsimd SWDGE); constants moved behind
  them. Cuts the 26us PE-idle head to ~13us.
- softmax denominator broadcast moved off the PE: reciprocal of the
  [1,512] ones-row directly from PSUM, then gpsimd partition_broadcast.
  (was a 13us fp32 ones-matmul on the tensor engine)
- V bias moved off the PE: bv pre-broadcast to [128, DG] once, added on
  the DVE during the V PSUM evacuation (was a K=1 matmul per V tile).
- wo accumulates all 4 head pairs into ONE PSUM tile -> single out
  tensor (half the evac + output-DMA bytes), output DMAs alternate
  sync/gpsimd queues.
- diagonal-slot exps merged into one ACT instruction where profitable.
"""

import sys

sys.path.insert(0, "/opt/trn_rl_repo")

import ml_dtypes
import numpy as np

import concourse.bass as bass  # noqa: F401
import concourse.bacc as bacc
import concourse.tile as tile
import concourse.mybir as mybir
from concourse.bass_utils import run_bass_kernel_spmd

F32 = mybir.dt.float32
F32R = mybir.dt.float32r
BF16 = mybir.dt.bfloat16
AF = mybir.ActivationFunctionType
BF = ml_dtypes.bfloat16

B, S, D = 4, 2048, 1024
H, DH = 16, 64
HG = 8  # heads per core
DG = HG * DH  # 512 out-dims per core

_PROGRAM = None
LAST_RESULTS = None  # for test.py introspection


def _build_program():
    nc = bacc.Bacc("TRN2", target_bir_lowering=False, debug=False)

    xq_t = nc.dram_tensor("xq_t", [4, 128, 8 * 512], BF16, kind="ExternalInput")
    xk_t = nc.dram_tensor("xk_t", [4, 128, 8 * 512], BF16, kind="ExternalInput")
    xv_t = nc.dram_tensor("xv_t", [16, 128, 8 * 128], BF16, kind="ExternalInput")
    wq_t = nc.dram_tensor("wq_t", [128, 8 * DG], BF16, kind="ExternalInput")
    wk_t = nc.dram_tensor("wk_t", [128, 8 * DG], BF16, kind="ExternalInput")
    wv_t = nc.dram_tensor("wv_t", [128, 8 * DG], BF16, kind="ExternalInput")
    wo_t = nc.dram_tensor("wo_t", [DG, D], BF16, kind="ExternalInput")
    bq_c = nc.dram_tensor("bq_c", [128, 4], F32, kind="ExternalInput")
    bk_c = nc.dram_tensor("bk_c", [128, 4], F32, kind="ExternalInput")
    bv_r = nc.dram_tensor("bv_r", [1, DG], BF16, kind="ExternalInput")
    ones_b = nc.dram_tensor("ones_b", [1, 128], BF16, kind="ExternalInput")
    ones_f = nc.dram_tensor("ones_f", [1, 128], BF16, kind="ExternalInput")
    ones8 = nc.dram_tensor("ones8", [128, 8], BF16, kind="ExternalInput")
    tri01 = nc.dram_tensor("tri01", [128, 128], BF16, kind="ExternalInput")
    out0 = nc.dram_tensor("out0", [S, D], BF16, kind="ExternalOutput")

    with tile.TileContext(nc) as tc:
        with (
            nc.allow_low_precision(reason="bf16 attention pipeline"),
            tc.tile_pool(name="persist", bufs=1) as pers,
        ):
            # ---- persistent tiles ----
            qT = [pers.tile([128, S], BF16, name=f"qT{i}") for i in range(4)]
            kT = [pers.tile([128, S], BF16, name=f"kT{i}") for i in range(4)]
            # v tiles: [128 s, 8 heads x (64 v + 1 ones)]
            vt = [pers.tile([128, HG * 65], BF16, name=f"v{i}") for i in range(16)]
            aout = [pers.tile([128, S], BF16, name=f"ao{i}") for i in range(4)]
            tri_sb = pers.tile([128, 128], BF16, name="tri_sb")
            ones_bf = pers.tile([1, 128], BF16, name="ones_bf")
            ones_fr = pers.tile([65, 128], BF16, name="ones_fr")
            bq_sb = pers.tile([128, 4], F32, name="bq")
            bk_sb = pers.tile([128, 4], F32, name="bk")
            bv_sb = pers.tile([1, DG], BF16, name="bv")
            bv_bc = pers.tile([128, DG], BF16, name="bv_bc")
            wq_sb = pers.tile([128, 8 * DG], BF16, name="wq_sb")
            wk_sb = pers.tile([128, 8 * DG], BF16, name="wk_sb")
            wv_sb = pers.tile([128, 8 * DG], BF16, name="wv_sb")
            wo_sb = [
                pers.tile([128, D], BF16, name=f"wo{c}") for c in range(4)
            ]

            # PSUM budget (8 banks): scores 2x[128,1024]=4, po 2x[65,512]=2,
            # pw 2x[128,512]=2 (proj evac / wo)
            pp = tc.alloc_tile_pool(name="pp", bufs=2, space="PSUM")
            with (
                tc.tile_pool(name="xb", bufs=5) as xp,
                tc.tile_pool(name="at", bufs=6) as ap_,
                tc.tile_pool(name="sm", bufs=6) as sm,
                tc.tile_pool(name="ob", bufs=4) as obp,
            ):
                # ---- startup DMAs: the first projection matmuls need only
                # wq/xq0 halves, so those go first (sync + gpsimd rings in
                # parallel); xk0/wk behind them (k-proj is emitted after all
                # of round-0 q-proj), wv last. Halves let the k8-accumulation
                # start as soon as the first 512KB lands.
                xq_big0 = xp.tile([128, 8 * 512], BF16, tag="xb", name="xq_big")
                xk_big0 = xp.tile([128, 8 * 512], BF16, tag="xb", name="xk_big")
                xv_early = []
                HC = 4 * 512
                nc.sync.dma_start(out=xq_big0[:, 0:HC], in_=xq_t[0, :, 0:HC])
                nc.gpsimd.dma_start(out=wq_sb[:, 0:HC], in_=wq_t[:, 0:HC])
                nc.sync.dma_start(out=xq_big0[:, HC:], in_=xq_t[0, :, HC:])
                nc.gpsimd.dma_start(out=wq_sb[:, HC:], in_=wq_t[:, HC:])
                nc.scalar.dma_start(out=bq_sb[:], in_=bq_c[:])
                nc.scalar.dma_start(out=bk_sb[:], in_=bk_c[:])
                nc.scalar.dma_start(out=xk_big0[:, 0:HC], in_=xk_t[0, :, 0:HC])
                nc.scalar.dma_start(out=xk_big0[:, HC:], in_=xk_t[0, :, HC:])
                nc.gpsimd.dma_start(out=wk_sb[:, 0:HC], in_=wk_t[:, 0:HC])
                nc.gpsimd.dma_start(out=wk_sb[:, HC:], in_=wk_t[:, HC:])
                nc.scalar.dma_start(out=wv_sb[:, 0:HC], in_=wv_t[:, 0:HC])
                nc.scalar.dma_start(out=wv_sb[:, HC:], in_=wv_t[:, HC:])
                # first four V activation chunks early on the gpsimd ring
                for s4 in range(4):
                    xvb = xp.tile([128, 8 * 128], BF16, tag="xvb", name="xv_big")
                    xv_early.append(xvb)
                    nc.gpsimd.dma_start(out=xvb[:], in_=xv_t[s4])
                nc.sync.dma_start(out=ones_bf[:], in_=ones_b[:])
                nc.sync.dma_start(out=ones_fr[64:65, :], in_=ones_f[:])
                nc.sync.dma_start(out=tri_sb[:], in_=tri01[:])
                nc.sync.dma_start(out=bv_sb[:], in_=bv_r[:])
                nc.gpsimd.partition_broadcast(bv_bc[:], bv_sb[:])
                # the per-head "ones" column of every v tile never changes:
                # fill them all up front (cheap DVE strided writes)
                ones8_sb = pers.tile([128, 8], BF16, name="ones8_sb")
                nc.sync.dma_start(out=ones8_sb[:], in_=ones8[:])
                for s in range(16):
                    v3s = vt[s].rearrange("p (h x) -> p h x", x=65)
                    nc.vector.tensor_copy(v3s[:, :, 64:65], ones8_sb[:].unsqueeze(2))
                # warm the ACT exp table (~2.7us) before the first real exp
                dummy = pers.tile([1, 128], BF16, name="dummy")
                nc.scalar.activation(dummy[:], ones_bf[:], AF.Exp)

                xqk_cache = {0: (xq_big0, xk_big0)}

                def emit_proj_qk(n, ms=(0, 1, 2, 3), which="qk"):
                    if n in xqk_cache:
                        xq_big, xk_big = xqk_cache[n]
                    else:
                        xq_big = xp.tile(
                            [128, 8 * 512], BF16, tag="xb", name="xq_big"
                        )
                        xk_big = xp.tile(
                            [128, 8 * 512], BF16, tag="xb", name="xk_big"
                        )
                        xqk_cache[n] = (xq_big, xk_big)
                        nc.sync.dma_start(out=xq_big[:], in_=xq_t[n])
                        nc.scalar.dma_start(out=xk_big[:], in_=xk_t[n])
                    parts = []
                    if "q" in which:
                        parts.append((wq_sb, xq_big, bq_sb, qT))
                    if "k" in which:
                        parts.append((wk_sb, xk_big, bk_sb, kT))
                    for m in ms:
                        for w_big, x_big, b_sb, dst in parts:
                            ps = pp.tile([128, 512], F32, tag="pw", bufs=2, name="ps1")
                            for k8 in range(8):
                                nc.tensor.matmul(
                                    ps[:],
                                    w_big[
                                        :,
                                        k8 * DG + m * 128 : k8 * DG + (m + 1) * 128,
                                    ],
                                    x_big[:, k8 * 512 : (k8 + 1) * 512],
                                    start=(k8 == 0),
                                    stop=(k8 == 7),
                                )
                            nc.scalar.activation(
                                dst[m][:, n * 512 : (n + 1) * 512],
                                ps[:],
                                AF.Identity,
                                bias=b_sb[:, m : m + 1],
                            )

                def emit_proj_v(s):
                    if s < 4:
                        xv_big = xv_early[s]
                    else:
                        xv_big = xp.tile(
                            [128, 8 * 128], BF16, tag="xvb", name="xv_big"
                        )
                        nc.sync.dma_start(out=xv_big[:], in_=xv_t[s])
                    ps = pp.tile([128, DG], F32, tag="pw", bufs=2, name="psv")
                    for k8 in range(8):
                        nc.tensor.matmul(
                            ps[:],
                            xv_big[:, k8 * 128 : (k8 + 1) * 128],
                            wv_sb[:, k8 * DG : (k8 + 1) * DG],
                            start=(k8 == 0),
                            stop=(k8 == 7),
                        )
                    v3 = vt[s].rearrange("p (h x) -> p h x", x=65)
                    nc.vector.tensor_add(
                        v3[:, :, 0:64],
                        ps[:].rearrange("p (h d) -> p h d", d=64),
                        bv_bc[:].rearrange("p (h d) -> p h d", d=64),
                    )

                def emit_attn(p, j, filler=None):
                    hA, hB = 2 * p, 2 * p + 1
                    nsk = 4 * j + 4
                    ps_oA = pp.tile([65, 512], F32, tag="po", bufs=2, name="ps_oA")
                    ps_oB = pp.tile([65, 512], F32, tag="po", bufs=2, name="ps_oB")
                    pending = []
                    # diagonal slots FIRST so the pair tail never waits on
                    # them; slot 4j has c0=0 so the first PV write still
                    # covers the full [0:512).
                    order = list(range(4 * j, nsk)) + list(range(0, 4 * j))
                    first_i = order[0]
                    for i in order:
                        koff = i - 4 * j
                        c0 = max(0, koff * 128) if koff >= 0 else 0
                        # both heads' scores in one 2-bank PSUM tile:
                        # head A at cols [0:512), head B at [512:1024).
                        # K=64 each, row-packed on the PE at row groups 0/64.
                        psAB = pp.tile([128, 1024], F32, tag="ps", bufs=2, name="psAB")
                        diag = koff >= 0
                        nc.tensor.matmul(
                            psAB[:, c0:512],
                            kT[p][0:64, i * 128 : (i + 1) * 128],
                            qT[p][0:64, j * 512 + c0 : (j + 1) * 512],
                            start=True,
                            stop=True,
                            tile_position=(0, 0),
                        )
                        nc.tensor.matmul(
                            psAB[:, 512 + c0 : 1024],
                            kT[p][64:128, i * 128 : (i + 1) * 128],
                            qT[p][64:128, j * 512 + c0 : (j + 1) * 512],
                            start=True,
                            stop=True,
                            tile_position=(64, 0),
                        )
                        # software-pipeline skew: retire PVs two iterations
                        # behind the exps so a PV never waits on its exp
                        while len(pending) >= 3:
                            pi, pc0, pat = pending.pop(0)
                            nc.tensor.matmul(
                                ps_oA[:, pc0:512],
                                vt[pi][:, hA * 65 : hA * 65 + 65],
                                pat[:, pc0:512],
                                start=(pi == first_i),
                                stop=False,
                            )
                            nc.tensor.matmul(
                                ps_oB[:, pc0:512],
                                vt[pi][:, hB * 65 : hB * 65 + 65],
                                pat[:, 512 + pc0 : 1024],
                                start=(pi == first_i),
                                stop=False,
                            )
                        atAB = ap_.tile([128, 1024], BF16, tag="at", name="atAB")
                        if c0 == 0:
                            nc.scalar.activation(
                                atAB[:], psAB[:], AF.Exp, scale=0.125
                            )
                        elif koff < 3 and not (p == 0 and j == 0):
                            # one merged exp over [c0:1024); the gap
                            # [512:512+c0) holds stale PSUM whose exp lands
                            # in an unread atAB region (skipped for the very
                            # first psAB allocations, which are uninit)
                            nc.scalar.activation(
                                atAB[:, c0:1024],
                                psAB[:, c0:1024],
                                AF.Exp,
                                scale=0.125,
                            )
                        else:
                            nc.scalar.activation(
                                atAB[:, c0:512], psAB[:, c0:512], AF.Exp, scale=0.125
                            )
                            nc.scalar.activation(
                                atAB[:, 512 + c0 : 1024],
                                psAB[:, 512 + c0 : 1024],
                                AF.Exp,
                                scale=0.125,
                            )
                        if diag:
                            # causal mask: zero the sub-diagonal triangle of
                            # the diag block after exp (was -1e9 on the PE)
                            nc.vector.tensor_mul(
                                atAB[:, c0 : c0 + 128],
                                atAB[:, c0 : c0 + 128],
                                tri_sb[:],
                            )
                            nc.vector.tensor_mul(
                                atAB[:, 512 + c0 : 512 + c0 + 128],
                                atAB[:, 512 + c0 : 512 + c0 + 128],
                                tri_sb[:],
                            )
                        pending.append((i, c0, atAB))
                        if filler:
                            emit_wo_item(*filler.pop(0))
                    while pending:
                        pi, pc0, pat = pending.pop(0)
                        last = not pending
                        nc.tensor.matmul(
                            ps_oA[:, pc0:512],
                            vt[pi][:, hA * 65 : hA * 65 + 65],
                            pat[:, pc0:512],
                            start=(pi == first_i),
                            stop=last,
                        )
                        nc.tensor.matmul(
                            ps_oB[:, pc0:512],
                            vt[pi][:, hB * 65 : hB * 65 + 65],
                            pat[:, 512 + pc0 : 1024],
                            start=(pi == first_i),
                            stop=last,
                        )
                    # normalize both heads; write bf16 straight into aout
                    for ps_o, hp in ((ps_oA, 0), (ps_oB, 64)):
                        den = sm.tile([65, 512], BF16, tag="den", bufs=2, name="den")
                        nc.vector.tensor_copy(den[64:65, :], ps_o[64:65, :])
                        ps_bc = pp.tile(
                            [64, 512], F32, tag="pw", bufs=2, name="ps_bc"
                        )
                        nc.tensor.matmul(
                            ps_bc[:],
                            ones_fr[64:65, 0:64],
                            den[64:65, :],
                            start=True,
                            stop=True,
                        )
                        rb = sm.tile([64, 512], F32, tag="rb", bufs=3, name="rb")
                        nc.vector.reciprocal_approx_fast(out=rb[:], in_=ps_bc[:])
                        dst = aout[p][hp : hp + 64, j * 512 : (j + 1) * 512]
                        if hp == 0:
                            nc.vector.tensor_mul(dst, ps_o[0:64, :], rb[:])
                        else:
                            tmp = sm.tile([64, 512], BF16, tag="tmp", bufs=3, name="tmp")
                            nc.vector.tensor_mul(tmp[:], ps_o[0:64, :], rb[:])
                            nc.gpsimd.dma_start(out=dst, in_=tmp[:])

                def emit_wo_item(s, n2, act_evac=False):
                    # accumulate all 4 head pairs into one PSUM tile
                    psw = pp.tile([128, 512], F32, tag="pw", bufs=2, name="psw")
                    for p4 in range(4):
                        nc.tensor.matmul(
                            psw[:],
                            aout[p4][:, s * 128 : (s + 1) * 128],
                            wo_sb[p4][:, n2 * 512 : (n2 + 1) * 512],
                            start=(p4 == 0),
                            stop=(p4 == 3),
                        )
                    ob = obp.tile([128, 512], BF16, tag="ob", name="ob")
                    if act_evac:
                        nc.scalar.copy(ob[:], psw[:])
                    else:
                        nc.vector.tensor_copy(ob[:], psw[:])
                    eng = nc.gpsimd if (2 * s + n2) % 2 else nc.sync
                    eng.dma_start(
                        out=out0[
                            s * 128 : (s + 1) * 128,
                            n2 * 512 : (n2 + 1) * 512,
                        ],
                        in_=ob[:],
                    )

                # ---- j-round synchronized emission ----
                # proj(n) and V(s) for round j+1 are emitted DURING round j
                # so their PE matmuls fill exp-paced attention bubbles and
                # their ACT evacuations land ahead of round j+1's exps.
                # output-projection items whose aout columns (s-tiles
                # 0..11 = j-chunks 0..2) are final before round 3 begins;
                # drained one per attention slot during round 3
                wo_fill = [(s, n2) for s in range(12) for n2 in range(2)]
                emit_proj_qk(0, ms=(0, 1), which="q")
                emit_proj_qk(0, ms=(0, 1), which="k")
                for s in range(4):
                    emit_proj_v(s)
                for j in range(4):
                    for p in range(4):
                        if p == 0 and j == 0:
                            emit_proj_qk(0, ms=(2, 3))
                        if p == 0 and j < 3:
                            emit_proj_qk(j + 1)
                        if p == 0 and j == 1:
                            # wo weights aren't needed until round 3
                            for c in range(4):
                                nc.gpsimd.dma_start(
                                    out=wo_sb[c][:],
                                    in_=wo_t[c * 128 : (c + 1) * 128, :],
                                )
                        # V tiles pulled forward: their evacuation runs on the
                        # (idle) vector engine, so unlike projections this
                        # fill adds no load to the exp-pacing scalar engine
                        if j == 0 and p == 1:
                            for s in range(4, 8):
                                emit_proj_v(s)
                        if j == 0 and p == 3:
                            for s in range(8, 12):
                                emit_proj_v(s)
                        if j == 1 and p == 1:
                            for s in range(12, 16):
                                emit_proj_v(s)
                        if j == 3:
                            emit_attn(p, j, filler=wo_fill)
                        else:
                            emit_attn(p, j)
                for item in wo_fill:
                    emit_wo_item(*item)
                for s in range(12, 16):
                    emit_wo_item(s, 0, act_evac=True)
                    emit_wo_item(s, 1)

            pp.release()

    nc.compile()
    return nc


def _make_in_maps(query, key, value, wq, bq, wk, bk, wv, bv, wo):
    f32 = np.float32
    ones_b = np.ones((1, 128), BF)
    ones8 = np.ones((128, 8), BF)
    # causal 0/1 mask for the diagonal block in scores_T layout:
    # rows=sk_local, cols=sq_local; valid iff sq_local >= sk_local
    tri01 = np.triu(np.ones((128, 128), np.float32)).astype(BF)

    wqT = np.asarray(wq, f32).T.astype(BF)  # [D, D] (d, dq)
    wkT = np.asarray(wk, f32).T.astype(BF)
    wvT = np.asarray(wv, f32).T.astype(BF)
    woT = np.asarray(wo, f32).T.astype(BF)  # [dv, D]

    def chunked_w(wT):
        # [1024, 512] (d, dq) -> [128, 8*512]: partition p holds d = k*128+p
        return np.ascontiguousarray(
            wT.reshape(8, 128, DG).transpose(1, 0, 2).reshape(128, 8 * DG)
        )

    def chunked_x(xT, n_chunks, cw):
        # [1024, S] -> [n_chunks, 128, 8*cw]: chunk n, partition p holds
        # rows d = k*128+p of columns [n*cw, (n+1)*cw)
        x3 = xT.reshape(8, 128, S)
        return np.ascontiguousarray(
            np.stack(
                [
                    x3[:, :, n * cw : (n + 1) * cw]
                    .transpose(1, 0, 2)
                    .reshape(128, 8 * cw)
                    for n in range(n_chunks)
                ]
            )
        )

    in_maps = []
    for c in range(8):
        b, g = c // 2, c % 2
        sl = slice(g * DG, (g + 1) * DG)
        in_maps.append(
            {
                "xq_t": chunked_x(np.asarray(query[b], f32).T.astype(BF), 4, 512),
                "xk_t": chunked_x(np.asarray(key[b], f32).T.astype(BF), 4, 512),
                "xv_t": chunked_x(np.asarray(value[b], f32).T.astype(BF), 16, 128),
                "wq_t": chunked_w(wqT[:, sl]),
                "wk_t": chunked_w(wkT[:, sl]),
                "wv_t": chunked_w(wvT[:, sl]),
                "wo_t": np.ascontiguousarray(woT[sl, :]),
                "bq_c": np.ascontiguousarray(
                    np.asarray(bq, f32)[sl].reshape(4, 128).T
                ),
                "bk_c": np.ascontiguousarray(
                    np.asarray(bk, f32)[sl].reshape(4, 128).T
                ),
                "bv_r": np.asarray(bv, f32)[sl].reshape(1, DG).astype(BF),
                "ones_b": ones_b,
                "ones_f": np.ones((1, 128), BF),
                "ones8": ones8,
                "tri01": tri01,
            }
        )
    return in_maps


def kernel(query, key, value, mask, wq, bq, wk, bk, wv, bv, wo, bo):
    global _PROGRAM, LAST_RESULTS
    if _PROGRAM is None:
        _PROGRAM = _build_program()
    nc = _PROGRAM
    in_maps = _make_in_maps(query, key, value, wq, bq, wk, bk, wv, bv, wo)

    res = run_bass_kernel_spmd(nc, in_maps, core_ids=list(range(8)))
    LAST_RESULTS = res

    f32 = np.float32
    out = np.empty((B, S, D), f32)
    for b in range(B):
        out[b] = np.asarray(res.results[2 * b]["out0"], f32) + np.asarray(
            res.results[2 * b + 1]["out0"], f32
        )
    out += np.asarray(bo, f32)[None, None, :]
    return out


# revision 17
# speedup vs baseline: 1.0590x; 1.0465x over previous
"""Multi-head attention (B=4, S=2048, D=1024, H=16, Dh=64) on 8 NeuronCores.

Sharding: core c handles batch b=c//2 and head-group g=c%2 (8 heads).
wq/wk/wv column-parallel, wo row-parallel; host sums the two partial
wo-products per batch and adds bo.

v3 changes vs v2 (346us):
- startup: big critical DMAs (xq0/xk0/wq) issued first across all three
  DMA queues (sync/scalar HWDGE + gpsimd SWDGE); constants moved behind
  them. Cuts the 26us PE-idle head to ~13us.
- softmax denominator broadcast moved off the PE: reciprocal of the
  [1,512] ones-row directly from PSUM, then gpsimd partition_broadcast.
  (was a 13us fp32 ones-matmul on the tensor engine)
- V bias moved off the PE: bv pre-broadcast to [128, DG] once, added on
  the DVE during the V PSUM evacuation (was a K=1 matmul per V tile).
- wo accumulates all 4 head pairs into ONE PSUM tile -> single out
  tensor (half the evac + output-DMA bytes), output DMAs alternate
  sync/gpsimd queues.
- diagonal-slot exps merged into one ACT instruction where profitable.
"""

import sys

sys.path.insert(0, "/opt/trn_rl_repo")

import ml_dtypes
import numpy as np

import concourse.bass as bass  # noqa: F401
import concourse.bacc as bacc
import concourse.tile as tile
import concourse.mybir as mybir
from concourse.bass_utils import run_bass_kernel_spmd

F32 = mybir.dt.float32
F32R = mybir.dt.float32r
BF16 = mybir.dt.bfloat16
AF = mybir.ActivationFunctionType
BF = ml_dtypes.bfloat16

B, S, D = 4, 2048, 1024
H, DH = 16, 64
HG = 8  # heads per core
DG = HG * DH  # 512 out-dims per core

_PROGRAM = None
LAST_RESULTS = None  # for test.py introspection


def _build_program():
    nc = bacc.Bacc("TRN2", target_bir_lowering=False, debug=False)

    xq_t = nc.dram_tensor("xq_t", [4, 128, 8 * 512], BF16, kind="ExternalInput")
    xk_t = nc.dram_tensor("xk_t", [4, 128, 8 * 512], BF16, kind="ExternalInput")
    xv_t = nc.dram_tensor("xv_t", [16, 128, 8 * 128], BF16, kind="ExternalInput")
    wq_t = nc.dram_tensor("wq_t", [128, 8 * DG], BF16, kind="ExternalInput")
    wk_t = nc.dram_tensor("wk_t", [128, 8 * DG], BF16, kind="ExternalInput")
    wv_t = nc.dram_tensor("wv_t", [128, 8 * DG], BF16, kind="ExternalInput")
    wo_t = nc.dram_tensor("wo_t", [DG, D], BF16, kind="ExternalInput")
    bq_c = nc.dram_tensor("bq_c", [128, 4], F32, kind="ExternalInput")
    bk_c = nc.dram_tensor("bk_c", [128, 4], F32, kind="ExternalInput")
    bv_r = nc.dram_tensor("bv_r", [1, DG], BF16, kind="ExternalInput")
    ones_b = nc.dram_tensor("ones_b", [1, 128], BF16, kind="ExternalInput")
    ones_f = nc.dram_tensor("ones_f", [1, 128], BF16, kind="ExternalInput")
    ones8 = nc.dram_tensor("ones8", [128, 8], BF16, kind="ExternalInput")
    maskaddT = nc.dram_tensor("maskaddT", [128, 128], BF16, kind="ExternalInput")
    ident = nc.dram_tensor("ident", [128, 128], BF16, kind="ExternalInput")
    out0 = nc.dram_tensor("out0", [S, D], BF16, kind="ExternalOutput")

    with tile.TileContext(nc) as tc:
        with (
            nc.allow_low_precision(reason="bf16 attention pipeline"),
            tc.tile_pool(name="persist", bufs=1) as pers,
        ):
            # ---- persistent tiles ----
            qT = [pers.tile([128, S], BF16, name=f"qT{i}") for i in range(4)]
            kT = [pers.tile([128, S], BF16, name=f"kT{i}") for i in range(4)]
            # v tiles: [128 s, 8 heads x (64 v + 1 ones)]
            vt = [pers.tile([128, HG * 65], BF16, name=f"v{i}") for i in range(16)]
            aout = [pers.tile([128, S], BF16, name=f"ao{i}") for i in range(4)]
            maskT_sb = pers.tile([128, 128], BF16, name="maskT_sb")
            ident_sb = pers.tile([128, 128], BF16, name="ident_sb")
            ones_bf = pers.tile([1, 128], BF16, name="ones_bf")
            ones_fr = pers.tile([65, 128], BF16, name="ones_fr")
            bq_sb = pers.tile([128, 4], F32, name="bq")
            bk_sb = pers.tile([128, 4], F32, name="bk")
            bv_sb = pers.tile([1, DG], BF16, name="bv")
            bv_bc = pers.tile([128, DG], BF16, name="bv_bc")
            wq_sb = pers.tile([128, 8 * DG], BF16, name="wq_sb")
            wk_sb = pers.tile([128, 8 * DG], BF16, name="wk_sb")
            wv_sb = pers.tile([128, 8 * DG], BF16, name="wv_sb")
            wo_sb = [
                pers.tile([128, D], BF16, name=f"wo{c}") for c in range(4)
            ]

            # PSUM budget (8 banks): scores 2x[128,1024]=4, po 2x[65,512]=2,
            # pw 2x[128,512]=2 (proj evac / wo)
            pp = tc.alloc_tile_pool(name="pp", bufs=2, space="PSUM")
            with (
                tc.tile_pool(name="xb", bufs=5) as xp,
                tc.tile_pool(name="at", bufs=6) as ap_,
                tc.tile_pool(name="sm", bufs=6) as sm,
                tc.tile_pool(name="ob", bufs=4) as obp,
            ):
                # ---- startup DMAs: the first projection matmuls need only
                # wq/xq0 halves, so those go first (sync + gpsimd rings in
                # parallel); xk0/wk behind them (k-proj is emitted after all
                # of round-0 q-proj), wv last. Halves let the k8-accumulation
                # start as soon as the first 512KB lands.
                xq_big0 = xp.tile([128, 8 * 512], BF16, tag="xb", name="xq_big")
                xk_big0 = xp.tile([128, 8 * 512], BF16, tag="xb", name="xk_big")
                xv_early = []
                HC = 4 * 512
                nc.sync.dma_start(out=xq_big0[:, 0:HC], in_=xq_t[0, :, 0:HC])
                nc.gpsimd.dma_start(out=wq_sb[:, 0:HC], in_=wq_t[:, 0:HC])
                nc.sync.dma_start(out=xq_big0[:, HC:], in_=xq_t[0, :, HC:])
                nc.gpsimd.dma_start(out=wq_sb[:, HC:], in_=wq_t[:, HC:])
                nc.scalar.dma_start(out=bq_sb[:], in_=bq_c[:])
                nc.scalar.dma_start(out=bk_sb[:], in_=bk_c[:])
                nc.scalar.dma_start(out=xk_big0[:, 0:HC], in_=xk_t[0, :, 0:HC])
                nc.scalar.dma_start(out=xk_big0[:, HC:], in_=xk_t[0, :, HC:])
                nc.gpsimd.dma_start(out=wk_sb[:, 0:HC], in_=wk_t[:, 0:HC])
                nc.gpsimd.dma_start(out=wk_sb[:, HC:], in_=wk_t[:, HC:])
                nc.scalar.dma_start(out=wv_sb[:, 0:HC], in_=wv_t[:, 0:HC])
                nc.scalar.dma_start(out=wv_sb[:, HC:], in_=wv_t[:, HC:])
                # first four V activation chunks early on the gpsimd ring
                for s4 in range(4):
                    xvb = xp.tile([128, 8 * 128], BF16, tag="xvb", name="xv_big")
                    xv_early.append(xvb)
                    nc.gpsimd.dma_start(out=xvb[:], in_=xv_t[s4])
                nc.sync.dma_start(out=ones_bf[:], in_=ones_b[:])
                nc.sync.dma_start(out=ones_fr[64:65, :], in_=ones_f[:])
                nc.sync.dma_start(out=maskT_sb[:], in_=maskaddT[:])
                nc.sync.dma_start(out=ident_sb[:], in_=ident[:])
                nc.sync.dma_start(out=bv_sb[:], in_=bv_r[:])
                nc.gpsimd.partition_broadcast(bv_bc[:], bv_sb[:])
                # the per-head "ones" column of every v tile never changes:
                # fill them all up front (cheap DVE strided writes)
                ones8_sb = pers.tile([128, 8], BF16, name="ones8_sb")
                nc.sync.dma_start(out=ones8_sb[:], in_=ones8[:])
                for s in range(16):
                    v3s = vt[s].rearrange("p (h x) -> p h x", x=65)
                    nc.vector.tensor_copy(v3s[:, :, 64:65], ones8_sb[:].unsqueeze(2))
                # warm the ACT exp table (~2.7us) before the first real exp
                dummy = pers.tile([1, 128], BF16, name="dummy")
                nc.scalar.activation(dummy[:], ones_bf[:], AF.Exp)

                xqk_cache = {0: (xq_big0, xk_big0)}

                def emit_proj_qk(n, ms=(0, 1, 2, 3), which="qk"):
                    if n in xqk_cache:
                        xq_big, xk_big = xqk_cache[n]
                    else:
                        xq_big = xp.tile(
                            [128, 8 * 512], BF16, tag="xb", name="xq_big"
                        )
                        xk_big = xp.tile(
                            [128, 8 * 512], BF16, tag="xb", name="xk_big"
                        )
                        xqk_cache[n] = (xq_big, xk_big)
                        nc.sync.dma_start(out=xq_big[:], in_=xq_t[n])
                        nc.scalar.dma_start(out=xk_big[:], in_=xk_t[n])
                    parts = []
                    if "q" in which:
                        parts.append((wq_sb, xq_big, bq_sb, qT))
                    if "k" in which:
                        parts.append((wk_sb, xk_big, bk_sb, kT))
                    for m in ms:
                        for w_big, x_big, b_sb, dst in parts:
                            ps = pp.tile([128, 512], F32, tag="pw", bufs=2, name="ps1")
                            for k8 in range(8):
                                nc.tensor.matmul(
                                    ps[:],
                                    w_big[
                                        :,
                                        k8 * DG + m * 128 : k8 * DG + (m + 1) * 128,
                                    ],
                                    x_big[:, k8 * 512 : (k8 + 1) * 512],
                                    start=(k8 == 0),
                                    stop=(k8 == 7),
                                )
                            nc.vector.tensor_scalar_add(
                                dst[m][:, n * 512 : (n + 1) * 512],
                                ps[:],
                                b_sb[:, m : m + 1],
                            )

                def emit_proj_v(s):
                    if s < 4:
                        xv_big = xv_early[s]
                    else:
                        xv_big = xp.tile(
                            [128, 8 * 128], BF16, tag="xvb", name="xv_big"
                        )
                        nc.sync.dma_start(out=xv_big[:], in_=xv_t[s])
                    ps = pp.tile([128, DG], F32, tag="pw", bufs=2, name="psv")
                    for k8 in range(8):
                        nc.tensor.matmul(
                            ps[:],
                            xv_big[:, k8 * 128 : (k8 + 1) * 128],
                            wv_sb[:, k8 * DG : (k8 + 1) * DG],
                            start=(k8 == 0),
                            stop=(k8 == 7),
                        )
                    v3 = vt[s].rearrange("p (h x) -> p h x", x=65)
                    nc.vector.tensor_add(
                        v3[:, :, 0:64],
                        ps[:].rearrange("p (h d) -> p h d", d=64),
                        bv_bc[:].rearrange("p (h d) -> p h d", d=64),
                    )

                def emit_attn(p, j, filler=None):
                    hA, hB = 2 * p, 2 * p + 1
                    nsk = 4 * j + 4
                    ps_oA = pp.tile([65, 512], F32, tag="po", bufs=2, name="ps_oA")
                    ps_oB = pp.tile([65, 512], F32, tag="po", bufs=2, name="ps_oB")
                    pending = []
                    # diagonal slots FIRST so the pair tail never waits on
                    # them; slot 4j has c0=0 so the first PV write still
                    # covers the full [0:512).
                    diag_slots = list(range(4 * j, nsk))
                    nd_slots = list(range(0, 4 * j))
                    order = []
                    for di, d in enumerate(diag_slots):
                        order.append(d)
                        if di < len(nd_slots):
                            order.append(nd_slots[di])
                    order += nd_slots[len(diag_slots) :]
                    first_i = order[0]
                    for i in order:
                        koff = i - 4 * j
                        c0 = max(0, koff * 128) if koff >= 0 else 0
                        # both heads' scores in one 2-bank PSUM tile:
                        # head A at cols [0:512), head B at [512:1024).
                        # K=64 each, row-packed on the PE at row groups 0/64.
                        psAB = pp.tile([128, 1024], BF16, tag="ps", bufs=2, name="psAB")
                        diag = koff >= 0
                        nc.tensor.matmul(
                            psAB[:, c0:512],
                            kT[p][0:64, i * 128 : (i + 1) * 128],
                            qT[p][0:64, j * 512 + c0 : (j + 1) * 512],
                            start=True,
                            stop=not diag,
                            tile_position=(0, 0),
                        )
                        nc.tensor.matmul(
                            psAB[:, 512 + c0 : 1024],
                            kT[p][64:128, i * 128 : (i + 1) * 128],
                            qT[p][64:128, j * 512 + c0 : (j + 1) * 512],
                            start=True,
                            stop=not diag,
                            tile_position=(64, 0),
                        )
                        if diag:
                            # add -1e9 below the diagonal ON the PE, inside
                            # the accumulation group: maskaddT.T @ I = maskadd
                            nc.tensor.matmul(
                                psAB[:, c0 : c0 + 128],
                                maskT_sb[:],
                                ident_sb[:],
                                start=False,
                                stop=True,
                            )
                            nc.tensor.matmul(
                                psAB[:, 512 + c0 : 512 + c0 + 128],
                                maskT_sb[:],
                                ident_sb[:],
                                start=False,
                                stop=True,
                            )
                        # software-pipeline skew: retire PVs two iterations
                        # behind the exps so a PV never waits on its exp
                        while len(pending) >= 3:
                            pi, pc0, pat = pending.pop(0)
                            nc.tensor.matmul(
                                ps_oA[:, pc0:512],
                                vt[pi][:, hA * 65 : hA * 65 + 65],
                                pat[:, pc0:512],
                                start=(pi == first_i),
                                stop=False,
                            )
                            nc.tensor.matmul(
                                ps_oB[:, pc0:512],
                                vt[pi][:, hB * 65 : hB * 65 + 65],
                                pat[:, 512 + pc0 : 1024],
                                start=(pi == first_i),
                                stop=False,
                            )
                        atAB = ap_.tile([128, 1024], BF16, tag="at", name="atAB")
                        if c0 == 0:
                            nc.scalar.activation(
                                atAB[:], psAB[:], AF.Exp, scale=0.125
                            )
                        elif koff < 3 and not (p == 0 and j == 0):
                            # one merged exp over [c0:1024); the gap
                            # [512:512+c0) holds stale PSUM whose exp lands
                            # in an unread atAB region (skipped for the very
                            # first psAB allocations, which are uninit)
                            nc.scalar.activation(
                                atAB[:, c0:1024],
                                psAB[:, c0:1024],
                                AF.Exp,
                                scale=0.125,
                            )
                        else:
                            nc.scalar.activation(
                                atAB[:, c0:512], psAB[:, c0:512], AF.Exp, scale=0.125
                            )
                            nc.scalar.activation(
                                atAB[:, 512 + c0 : 1024],
                                psAB[:, 512 + c0 : 1024],
                                AF.Exp,
                                scale=0.125,
                            )
                        pending.append((i, c0, atAB))
                        if filler:
                            emit_wo_item(*filler.pop(0))
                    while pending:
                        pi, pc0, pat = pending.pop(0)
                        last = not pending
                        nc.tensor.matmul(
                            ps_oA[:, pc0:512],
                            vt[pi][:, hA * 65 : hA * 65 + 65],
                            pat[:, pc0:512],
                            start=(pi == first_i),
                            stop=last,
                        )
                        nc.tensor.matmul(
                            ps_oB[:, pc0:512],
                            vt[pi][:, hB * 65 : hB * 65 + 65],
                            pat[:, 512 + pc0 : 1024],
                            start=(pi == first_i),
                            stop=last,
                        )
                    # normalize both heads; write bf16 straight into aout
                    for ps_o, hp in ((ps_oA, 0), (ps_oB, 64)):
                        den = sm.tile([65, 512], BF16, tag="den", bufs=2, name="den")
                        nc.vector.tensor_copy(den[64:65, :], ps_o[64:65, :])
                        ps_bc = pp.tile(
                            [64, 512], F32, tag="pw", bufs=2, name="ps_bc"
                        )
                        nc.tensor.matmul(
                            ps_bc[:],
                            ones_fr[64:65, 0:64],
                            den[64:65, :],
                            start=True,
                            stop=True,
                        )
                        rb = sm.tile([64, 512], F32, tag="rb", bufs=3, name="rb")
                        nc.vector.reciprocal_approx_fast(out=rb[:], in_=ps_bc[:])
                        dst = aout[p][hp : hp + 64, j * 512 : (j + 1) * 512]
                        if hp == 0:
                            nc.vector.tensor_mul(dst, ps_o[0:64, :], rb[:])
                        else:
                            tmp = sm.tile(
                                [64, 512], BF16, tag="tmp", bufs=3, name="tmp"
                            )
                            nc.vector.tensor_mul(tmp[:], ps_o[0:64, :], rb[:])
                            nc.gpsimd.dma_start(out=dst, in_=tmp[:])

                def emit_wo_item(s, n2, act_evac=False):
                    # accumulate all 4 head pairs into one PSUM tile
                    psw = pp.tile([128, 512], F32, tag="pw", bufs=2, name="psw")
                    for p4 in range(4):
                        nc.tensor.matmul(
                            psw[:],
                            aout[p4][:, s * 128 : (s + 1) * 128],
                            wo_sb[p4][:, n2 * 512 : (n2 + 1) * 512],
                            start=(p4 == 0),
                            stop=(p4 == 3),
                        )
                    ob = obp.tile([128, 512], BF16, tag="ob", name="ob")
                    if act_evac:
                        nc.scalar.copy(ob[:], psw[:])
                    else:
                        nc.vector.tensor_copy(ob[:], psw[:])
                    eng = nc.gpsimd if (2 * s + n2) % 2 else nc.sync
                    eng.dma_start(
                        out=out0[
                            s * 128 : (s + 1) * 128,
                            n2 * 512 : (n2 + 1) * 512,
                        ],
                        in_=ob[:],
                    )

                # ---- j-round synchronized emission ----
                # proj(n) and V(s) for round j+1 are emitted DURING round j
                # so their PE matmuls fill exp-paced attention bubbles and
                # their ACT evacuations land ahead of round j+1's exps.
                # output-projection items whose aout columns (s-tiles
                # 0..11 = j-chunks 0..2) are final before round 3 begins;
                # drained one per attention slot during round 3
                wo_fill = [(s, n2) for s in range(12) for n2 in range(2)]
                emit_proj_qk(0, ms=(0, 1), which="q")
                emit_proj_qk(0, ms=(0, 1), which="k")
                for s in range(4):
                    emit_proj_v(s)
                for j in range(4):
                    for p in range(4):
                        if p == 0 and j == 0:
                            emit_proj_qk(0, ms=(2, 3))
                        if j < 3:
                            # spread next-round projection prefetch across the
                            # four head-pairs so every (p,j) boundary has
                            # dependency-free PE filler work
                            emit_proj_qk(j + 1, ms=(p,))
                        if p == 0 and j == 1:
                            # wo weights aren't needed until round 3
                            for c in range(4):
                                nc.gpsimd.dma_start(
                                    out=wo_sb[c][:],
                                    in_=wo_t[c * 128 : (c + 1) * 128, :],
                                )
                        # V tiles pulled forward: their evacuation runs on the
                        # (idle) vector engine, so unlike projections this
                        # fill adds no load to the exp-pacing scalar engine
                        if j == 0 and p == 1:
                            for s in range(4, 8):
                                emit_proj_v(s)
                        if j == 0 and p == 3:
                            for s in range(8, 12):
                                emit_proj_v(s)
                        if j == 1 and p == 1:
                            for s in range(12, 16):
                                emit_proj_v(s)
                        if j == 3:
                            emit_attn(p, j, filler=wo_fill)
                        else:
                            emit_attn(p, j)
                for item in wo_fill:
                    emit_wo_item(*item)
                for s in range(12, 16):
                    emit_wo_item(s, 0, act_evac=True)
                    emit_wo_item(s, 1)

            pp.release()

    nc.compile()
    return nc


def _make_in_maps(query, key, value, wq, bq, wk, bk, wv, bv, wo):
    f32 = np.float32
    ones_b = np.ones((1, 128), BF)
    ones8 = np.ones((128, 8), BF)
    # causal mask add-block in scores_T layout: rows=sk_local, cols=sq_local;
    # valid iff sq_local >= sk_local. Shipped TRANSPOSED: the kernel adds it
    # on the PE as maskaddT.T @ I inside the scores accumulation group.
    maskadd = np.where(
        np.triu(np.ones((128, 128), bool)), f32(0), f32(-1.0e9)
    ).astype(f32)
    maskaddT = np.ascontiguousarray(maskadd.T).astype(BF)
    ident = np.eye(128, dtype=BF)

    wqT = np.asarray(wq, f32).T.astype(BF)  # [D, D] (d, dq)
    wkT = np.asarray(wk, f32).T.astype(BF)
    wvT = np.asarray(wv, f32).T.astype(BF)
    woT = np.asarray(wo, f32).T.astype(BF)  # [dv, D]

    def chunked_w(wT):
        # [1024, 512] (d, dq) -> [128, 8*512]: partition p holds d = k*128+p
        return np.ascontiguousarray(
            wT.reshape(8, 128, DG).transpose(1, 0, 2).reshape(128, 8 * DG)
        )

    def chunked_x(xT, n_chunks, cw):
        # [1024, S] -> [n_chunks, 128, 8*cw]: chunk n, partition p holds
        # rows d = k*128+p of columns [n*cw, (n+1)*cw)
        x3 = xT.reshape(8, 128, S)
        return np.ascontiguousarray(
            np.stack(
                [
                    x3[:, :, n * cw : (n + 1) * cw]
                    .transpose(1, 0, 2)
                    .reshape(128, 8 * cw)
                    for n in range(n_chunks)
                ]
            )
        )

    in_maps = []
    for c in range(8):
        b, g = c // 2, c % 2
        sl = slice(g * DG, (g + 1) * DG)
        in_maps.append(
            {
                "xq_t": chunked_x(np.asarray(query[b], f32).T.astype(BF), 4, 512),
                "xk_t": chunked_x(np.asarray(key[b], f32).T.astype(BF), 4, 512),
                "xv_t": chunked_x(np.asarray(value[b], f32).T.astype(BF), 16, 128),
                "wq_t": chunked_w(wqT[:, sl]),
                "wk_t": chunked_w(wkT[:, sl]),
                "wv_t": chunked_w(wvT[:, sl]),
                "wo_t": np.ascontiguousarray(woT[sl, :]),
                "bq_c": np.ascontiguousarray(
                    np.asarray(bq, f32)[sl].reshape(4, 128).T
                ),
                "bk_c": np.ascontiguousarray(
                    np.asarray(bk, f32)[sl].reshape(4, 128).T
                ),
                "bv_r": np.asarray(bv, f32)[sl].reshape(1, DG).astype(BF),
                "ones_b": ones_b,
                "ones_f": np.ones((1, 128), BF),
                "ones8": ones8,
                "maskaddT": maskaddT,
                "ident": ident,
            }
        )
    return in_maps


def kernel(query, key, value, mask, wq, bq, wk, bk, wv, bv, wo, bo):
    global _PROGRAM, LAST_RESULTS
    if _PROGRAM is None:
        _PROGRAM = _build_program()
    nc = _PROGRAM
    in_maps = _make_in_maps(query, key, value, wq, bq, wk, bk, wv, bv, wo)

    res = run_bass_kernel_spmd(nc, in_maps, core_ids=list(range(8)))
    LAST_RESULTS = res

    f32 = np.float32
    out = np.empty((B, S, D), f32)
    for b in range(B):
        out[b] = np.asarray(res.results[2 * b]["out0"], f32) + np.asarray(
            res.results[2 * b + 1]["out0"], f32
        )
    out += np.asarray(bo, f32)[None, None, :]
    return out
